# revision 8
# baseline (speedup 1.0000x reference)
"""NT-Xent / SimCLR contrastive loss on 8 Trainium2 NeuronCores.

Strategy (data-parallel over rows of the concatenated representations):
  - Host: reps = concat(z_i, z_j) -> [8192, 512], cast bf16. Core i receives
    reps rolled by -1024*i rows so its 1024 rows sit at rows 0..1023 (SPMD
    program identical on every core; positives land at col = row + 4096).
  - Device (per core), fp8 pipeline:
      phase A (per 2048-row group g): load bf16 rows, 1/||row|| via fused
        square+rowsum (DVE) and exp(-0.5*ln(n2) + ln C) (ACT, one table set),
        scale rows by C/||row|| into fp8e4 (DVE), then SBUF->SBUF xbar DMA
        transposes of [128,128] uint16 blocks (fp8 PAIRS) into repsT8.
        Feature pairing: u16 element q of a row = features (2q, 2q+1); after
        transpose, partition p holds the fp8 pair = DoubleRow planes 0/1.
      phase B (per group nb, m-block): sim slice via DoubleRow fp8 matmuls
        (K=512 as 2 packed 256-chunks, 2x PE rate) into [128, 2048] PSUM;
        ACT computes exp((2/C^2)*sim) with fused row-sum; DVE extracts
        self/positive diagonals with an identity mask + fused reduce.
      A(g) and B(nb=g-1) are interleaved so ACT/PE/DVE/DMA queues pipeline.
      epilogue: denom = rowsum - exp(2*sim_self); partial row loss is
        ln(denom) - 2*pos; partition-sum via a ones-matmul; scalar out.
  - Host: loss = sum(core partials) / 8192.
"""

import math
import sys
import threading
from unittest import mock

sys.path.insert(0, "/opt/trn_rl_repo")

import numpy as np  # noqa: E402
import ml_dtypes  # noqa: E402

import concourse.tile as tile  # noqa: E402
from concourse import bacc, mybir  # noqa: E402
from concourse.bass_utils import run_bass_kernel_spmd  # noqa: E402
from concourse.hw_specs import get_activation_tables  # noqa: E402
from concourse.masks import make_identity  # noqa: E402
from contextlib import ExitStack  # noqa: E402

P = 128
D = 512
TWO_N = 8192
N_CORES = 8
ROWS_PER_CORE = TWO_N // N_CORES  # 1024
T_INV = 2.0  # 1 / temperature (0.5)

NB = 4  # column groups (= row groups in phase A)
CB = TWO_N // NB  # 2048 columns per group / rows per group
TPG = CB // P  # 16 [128, 512] row tiles per group
MB = ROWS_PER_CORE // P  # 8 m-blocks of 128 rows per core
NS = CB // 512  # 4 matmul sub-columns of 512 (one PSUM bank each)
KK = 2  # DoubleRow packed K chunks (256 features each)
LOAD_SPLIT = 4  # sub-DMAs per group load (pipeline the first tiles)

C_SCALE = 512.0  # fp8 range scale; rows stored as C*x/||x||
LN_C = math.log(C_SCALE)
SIM_SCALE = T_INV / (C_SCALE * C_SCALE)  # exact power of two: 2^-17

FP32 = mybir.dt.float32
BF16 = mybir.dt.bfloat16
FP8 = mybir.dt.float8e4
U16 = mybir.dt.uint16
AF = mybir.ActivationFunctionType
ALU = mybir.AluOpType
AX = mybir.AxisListType
DR = mybir.MatmulPerfMode.DoubleRow


def _filtered_activation_tables(arch):
    """Steer every Exp/Ln/Copy activation to the one table set containing
    both Exp and Ln, so the table-load pass cannot thrash between sets."""
    tables = get_activation_tables(arch)
    target = None
    for name, funcs in tables.items():
        if AF.Exp in funcs and AF.Ln in funcs:
            target = name
            break
    if target is None:
        return tables
    steer = {AF.Exp, AF.Ln, AF.Copy, AF.Identity}
    return {
        name: (funcs if name == target else funcs - steer)
        for name, funcs in tables.items()
    }


def _build_kernel():
    nc = bacc.Bacc("TRN2", target_bir_lowering=False, debug=False,
                   num_devices=N_CORES)
    reps = nc.dram_tensor("reps", [TWO_N, D], BF16, kind="ExternalInput").ap()
    out = nc.dram_tensor("out", [1, 1], FP32, kind="ExternalOutput").ap()

    with tile.TileContext(nc) as tc, ExitStack() as ctx:
        rows_pool = ctx.enter_context(tc.tile_pool(name="rows", bufs=2))
        normed_pool = ctx.enter_context(tc.tile_pool(name="normed", bufs=2))
        sq_pool = ctx.enter_context(tc.tile_pool(name="sq", bufs=2))
        stats_pool = ctx.enter_context(tc.tile_pool(name="stats", bufs=1))
        repsT_pool = ctx.enter_context(tc.tile_pool(name="repsT", bufs=1))
        psum_pool = ctx.enter_context(
            tc.tile_pool(name="psum", bufs=2, space="PSUM"))
        exp_pool = ctx.enter_context(tc.tile_pool(name="exp", bufs=2))
        junk_pool = ctx.enter_context(tc.tile_pool(name="junk", bufs=2))
        epi_pool = ctx.enter_context(tc.tile_pool(name="epi", bufs=1))

        # --- constants -----------------------------------------------------
        ident = stats_pool.tile([P, P], FP32, tag="ident", name="ident")
        make_identity(nc, ident[:])
        ones = stats_pool.tile([P, 1], FP32, tag="ones", name="ones")
        nc.gpsimd.memset(ones[:], 1.0)

        # accumulators for the main loop
        rs_all = stats_pool.tile([P, MB * NB], FP32, tag="rs", name="rs_all")
        e_self = stats_pool.tile([P, MB], FP32, tag="eself", name="e_self")
        pos = stats_pool.tile([P, MB], FP32, tag="pos", name="pos")

        # repsT8[kk][g]: [128, 2048] u16 - partition p = feature pair
        # (kk*256 + 2p, kk*256 + 2p + 1) packed as 2 fp8 bytes; column c =
        # global (rolled) row g*2048 + c. fp8 bitcast views give the
        # DoubleRow [128, 2, N] operand APs directly.
        repsT8 = [[repsT_pool.tile([P, CB], U16, tag=f"rT{kk}_{g}",
                                   name=f"repsT8_{kk}_{g}")
                   for g in range(NB)]
                  for kk in range(KK)]

        def rhs_ap(kk, g, ns):
            v = repsT8[kk][g][:].bitcast(FP8).rearrange(
                "p (n two) -> p two n", two=2)
            return v[:, :, ns * 512:(ns + 1) * 512]

        # repsT0[kk]: [128, 2*1024] fp8, plane-slab layout (plane i at cols
        # i*1024..) — LDWEIGHTS rejects the byte-interleaved stride-2 AP, so
        # deinterleave the core's own 1024 columns for the stationary side.
        repsT0_pool = ctx.enter_context(tc.tile_pool(name="repsT0", bufs=1))
        repsT0 = [repsT0_pool.tile([P, 2 * ROWS_PER_CORE], FP8,
                                   tag=f"rT0_{kk}", name=f"repsT0_{kk}")
                  for kk in range(KK)]

        def lhs_ap(kk, m):
            v = repsT0[kk][:].rearrange("p (two m) -> p two m", two=2)
            return v[:, :, m * P:(m + 1) * P]

        # --- issue all row loads up front (DMA runs ahead) -----------------
        rows_g = []
        for g in range(NB):
            rg = rows_pool.tile([P, TPG * D], BF16, tag=f"rows{g}", bufs=1,
                                name=f"rows_{g}")
            rows_g.append(rg)
        TSUB = TPG // LOAD_SPLIT  # row tiles per sub-DMA
        for g in range(NB):
            for s in range(LOAD_SPLIT):
                r0 = g * CB + s * TSUB * P
                src = reps[r0:r0 + TSUB * P, :].rearrange(
                    "(t p) d -> p t d", p=P)
                dst = rows_g[g][:, s * TSUB * D:(s + 1) * TSUB * D].rearrange(
                    "p (t d) -> p t d", d=D)
                nc.sync.dma_start(out=dst, in_=src)

        def phase_a(g):
            n2 = stats_pool.tile([P, TPG], FP32, tag="n2", bufs=2,
                                 name=f"n2_{g}")
            for t in range(TPG):
                sq = sq_pool.tile([P, D], BF16, tag="sq", name=f"sq_{g}_{t}")
                rt = rows_g[g][:, t * D:(t + 1) * D]
                nc.vector.scalar_tensor_tensor(
                    out=sq[:], in0=rt, scalar=1.0, in1=rt,
                    op0=ALU.mult, op1=ALU.mult, accum_out=n2[:, t:t + 1])
            # inv = C * n2 ** -0.5 = exp(-0.5 * ln(n2 / C^2)); Ln+Exp share
            # one ACT table set (forced via _filtered_activation_tables).
            lnn = stats_pool.tile([P, TPG], FP32, tag="lnn", bufs=2,
                                  name=f"lnn_{g}")
            nc.scalar.activation(lnn[:], n2[:], AF.Ln,
                                 scale=1.0 / (C_SCALE * C_SCALE))
            inv = stats_pool.tile([P, TPG], FP32, tag="inv", bufs=2,
                                  name=f"inv_{g}")
            nc.scalar.activation(inv[:], lnn[:], AF.Exp, scale=-0.5)

            normed8 = normed_pool.tile([P, TPG * D], FP8, tag="normed",
                                       name=f"normed_{g}")
            for t in range(TPG):
                nc.vector.tensor_scalar_mul(
                    normed8[:, t * D:(t + 1) * D],
                    rows_g[g][:, t * D:(t + 1) * D], inv[:, t:t + 1])
            # SBUF->SBUF xbar transposes of [128,128] u16 blocks (fp8 pairs).
            nview16 = normed8[:].bitcast(U16)  # [P, TPG*256]
            for t in range(TPG):
                for kk in range(KK):
                    src = nview16[:, t * 256 + kk * P:t * 256 + (kk + 1) * P]
                    nc.sync.dma_start_transpose(
                        repsT8[kk][g][:, t * P:(t + 1) * P], src)

        def phase_b(nb):
            for m in range(MB):
                ps = psum_pool.tile([P, CB], FP32, tag="ps",
                                    name=f"ps_{nb}_{m}")
                for ns in range(NS):
                    for kk in range(KK):
                        nc.tensor.matmul(
                            ps[:, ns * 512:(ns + 1) * 512],
                            lhsT=lhs_ap(kk, m),
                            rhs=rhs_ap(kk, nb, ns),
                            start=(kk == 0), stop=(kk == KK - 1),
                            perf_mode=DR)
                et = exp_pool.tile([P, CB], BF16, tag="et",
                                   name=f"et_{nb}_{m}")
                nc.scalar.activation(
                    et[:], ps[:], AF.Exp, scale=SIM_SCALE,
                    accum_out=rs_all[:, m * NB + nb:m * NB + nb + 1])
                if nb == 0:
                    # self-similarity diagonal: col m*128+j for psum row j
                    junk = junk_pool.tile([P, P], FP32, tag="junk",
                                          name=f"junk_s_{m}")
                    nc.vector.scalar_tensor_tensor(
                        out=junk[:], in0=et[:, m * P:(m + 1) * P],
                        scalar=1.0, in1=ident[:],
                        op0=ALU.mult, op1=ALU.mult,
                        accum_out=e_self[:, m:m + 1])
                if nb == 2:
                    # positive diagonal: global col 4096+row -> group 2,
                    # in-group col m*128+j. ps holds C^2 * sim.
                    junk = junk_pool.tile([P, P], FP32, tag="junk",
                                          name=f"junk_p_{m}")
                    nc.vector.scalar_tensor_tensor(
                        out=junk[:], in0=ps[:, m * P:(m + 1) * P],
                        scalar=1.0, in1=ident[:],
                        op0=ALU.mult, op1=ALU.mult,
                        accum_out=pos[:, m:m + 1])

        # interleave: A(g0) B(0) | A(g1) B(1) | ... so every engine queue
        # pipelines (ACT: ln/exp(g) then 8 exps(nb); PE streams while the
        # next group loads/normalizes/transposes).
        for g in range(NB):
            phase_a(g)
            if g == 0:
                for kk in range(KK):
                    iv = repsT8[kk][0][:].bitcast(FP8).rearrange(
                        "p (n two) -> p two n", two=2)
                    for i in range(2):
                        nc.vector.tensor_copy(
                            repsT0[kk][:, i * ROWS_PER_CORE:
                                       (i + 1) * ROWS_PER_CORE],
                            iv[:, i, :ROWS_PER_CORE])
            phase_b(g)

        # --- epilogue ------------------------------------------------------
        sums = epi_pool.tile([P, MB], FP32, tag="sums", name="sums")
        nc.vector.tensor_reduce(
            sums[:], rs_all[:].rearrange("p (m b) -> p m b", b=NB),
            axis=AX.X, op=ALU.add)
        denom = epi_pool.tile([P, MB], FP32, tag="denom", name="denom")
        nc.vector.tensor_sub(denom[:], sums[:], e_self[:])
        ld = epi_pool.tile([P, MB], FP32, tag="ld", name="ld")
        nc.scalar.activation(ld[:], denom[:], AF.Ln)
        # partial = ld - (2/C^2)*pos = (pos * -SIM_SCALE) + ld
        part = epi_pool.tile([P, MB], FP32, tag="part", name="part")
        nc.vector.scalar_tensor_tensor(
            out=part[:], in0=pos[:], scalar=-SIM_SCALE, in1=ld[:],
            op0=ALU.mult, op1=ALU.add)
        rowtot = epi_pool.tile([P, 1], FP32, tag="rowtot", name="rowtot")
        nc.vector.tensor_reduce(rowtot[:], part[:], axis=AX.X, op=ALU.add)
        pfin = psum_pool.tile([P, CB], FP32, tag="ps", name="pfin")
        nc.tensor.matmul(pfin[:1, :1], lhsT=ones[:], rhs=rowtot[:])
        out_sb = epi_pool.tile([1, 1], FP32, tag="osb", name="out_sb")
        nc.vector.tensor_copy(out_sb[:], pfin[:1, :1])
        nc.sync.dma_start(out=out[:, :], in_=out_sb[:])

    with mock.patch("concourse.bacc.get_activation_tables",
                    _filtered_activation_tables):
        nc.compile()
    return nc


_CACHE_LOCK = threading.Lock()
_CACHED_NC = None


def _get_nc():
    global _CACHED_NC
    with _CACHE_LOCK:
        if _CACHED_NC is None:
            _CACHED_NC = _build_kernel()
        return _CACHED_NC


def _run(inputs, trace=False):
    z_i = np.asarray(inputs["z_i"], dtype=np.float32)
    z_j = np.asarray(inputs["z_j"], dtype=np.float32)
    reps = np.concatenate([z_i, z_j], axis=0).astype(ml_dtypes.bfloat16)
    in_maps = [
        {"reps": np.ascontiguousarray(
            np.roll(reps, -ROWS_PER_CORE * i, axis=0))}
        for i in range(N_CORES)
    ]
    nc = _get_nc()
    res = run_bass_kernel_spmd(nc, in_maps, list(range(N_CORES)), trace=trace)
    partials = [float(res.results[i]["out"][0, 0]) for i in range(N_CORES)]
    loss = np.float32(np.sum(np.asarray(partials, dtype=np.float64)) / TWO_N)
    return loss, res


def kernel(**inputs):
    loss, _ = _run(inputs, trace=False)
    return np.asarray(loss, dtype=np.float32)


# revision 15
# speedup vs baseline: 1.6288x; 1.6288x over previous
"""NT-Xent / SimCLR contrastive loss on 8 Trainium2 NeuronCores.

Strategy (data-parallel over rows of the concatenated representations):
  - Host: reps = concat(z_i, z_j) -> [8192, 512], cast bf16. Core i receives
    reps rolled by -1024*i rows so its 1024 rows sit at rows 0..1023 (SPMD
    program identical on every core; positives land at col = row + 4096).
  - Device (per core), fp8 pipeline:
      phase A (per 2048-row group g): load bf16 rows, 1/||row|| via fused
        square+rowsum (DVE) and exp(-0.5*ln(n2) + ln C) (ACT, one table set),
        scale rows by C/||row|| into fp8e4 (DVE), then SBUF->SBUF xbar DMA
        transposes of [128,128] uint16 blocks (fp8 PAIRS) into repsT8.
        Feature pairing: u16 element q of a row = features (2q, 2q+1); after
        transpose, partition p holds the fp8 pair = DoubleRow planes 0/1.
      phase B (per group nb, m-block): sim slice via DoubleRow fp8 matmuls
        (K=512 as 2 packed 256-chunks, 2x PE rate) into [128, 2048] PSUM;
        ACT computes exp((2/C^2)*sim) with fused row-sum; DVE extracts
        self/positive diagonals with an identity mask + fused reduce.
      A(g) and B(nb=g-1) are interleaved so ACT/PE/DVE/DMA queues pipeline.
      epilogue: denom = rowsum - exp(2*sim_self); partial row loss is
        ln(denom) - 2*pos; partition-sum via a ones-matmul; scalar out.
  - Host: loss = sum(core partials) / 8192.
"""

import math
import sys
import threading
from unittest import mock

sys.path.insert(0, "/opt/trn_rl_repo")

import numpy as np  # noqa: E402
import ml_dtypes  # noqa: E402

import concourse.tile as tile  # noqa: E402
from concourse import bacc, mybir  # noqa: E402
from concourse.bass_utils import run_bass_kernel_spmd  # noqa: E402
from concourse.hw_specs import get_activation_tables  # noqa: E402
from concourse.masks import make_identity  # noqa: E402
from contextlib import ExitStack  # noqa: E402

P = 128
D = 512
TWO_N = 8192
N_CORES = 8
ROWS_PER_CORE = TWO_N // N_CORES  # 1024
T_INV = 2.0  # 1 / temperature (0.5)

NB = 4  # column groups (= row groups in phase A)
CB = TWO_N // NB  # 2048 columns per group / rows per group
TPG = CB // P  # 16 [128, 512] row tiles per group
MB = ROWS_PER_CORE // P  # 8 m-blocks of 128 rows per core
NS = CB // 512  # 4 matmul sub-columns of 512 (one PSUM bank each)
KK = 2  # DoubleRow packed K chunks (256 features each)
LOAD_SPLIT = 4  # sub-DMAs per group load (pipeline the first tiles)

C_SCALE = 512.0  # fp8 range scale; rows stored as C*x/||x||
LN_C = math.log(C_SCALE)
SIM_SCALE = T_INV / (C_SCALE * C_SCALE)  # exact power of two: 2^-17

FP32 = mybir.dt.float32
BF16 = mybir.dt.bfloat16
FP8 = mybir.dt.float8e4
U16 = mybir.dt.uint16
AF = mybir.ActivationFunctionType
ALU = mybir.AluOpType
AX = mybir.AxisListType
DR = mybir.MatmulPerfMode.DoubleRow


def _filtered_activation_tables(arch):
    """Steer every Exp/Ln/Copy activation to the one table set containing
    both Exp and Ln, so the table-load pass cannot thrash between sets."""
    tables = get_activation_tables(arch)
    target = None
    for name, funcs in tables.items():
        if AF.Exp in funcs and AF.Ln in funcs:
            target = name
            break
    if target is None:
        return tables
    steer = {AF.Exp, AF.Ln, AF.Copy, AF.Identity}
    return {
        name: (funcs if name == target else funcs - steer)
        for name, funcs in tables.items()
    }


def _build_kernel():
    nc = bacc.Bacc("TRN2", target_bir_lowering=False, debug=False,
                   num_devices=N_CORES)
    reps = nc.dram_tensor("reps", [TWO_N, D], BF16, kind="ExternalInput").ap()
    out = nc.dram_tensor("out", [1, 1], FP32, kind="ExternalOutput").ap()

    with tile.TileContext(nc) as tc, ExitStack() as ctx:
        rows_pool = ctx.enter_context(tc.tile_pool(name="rows", bufs=2))
        normed_pool = ctx.enter_context(tc.tile_pool(name="normed", bufs=2))
        sq_pool = ctx.enter_context(tc.tile_pool(name="sq", bufs=2))
        stats_pool = ctx.enter_context(tc.tile_pool(name="stats", bufs=1))
        repsT_pool = ctx.enter_context(tc.tile_pool(name="repsT", bufs=1))
        dram_pool = ctx.enter_context(
            tc.tile_pool(name="scratch", bufs=KK * NB, space="DRAM"))
        psum_pool = ctx.enter_context(
            tc.tile_pool(name="psum", bufs=2, space="PSUM"))
        exp_pool = ctx.enter_context(tc.tile_pool(name="exp", bufs=2))
        junk_pool = ctx.enter_context(tc.tile_pool(name="junk", bufs=2))
        epi_pool = ctx.enter_context(tc.tile_pool(name="epi", bufs=1))

        # --- constants -----------------------------------------------------
        ident = stats_pool.tile([P, P], FP32, tag="ident", name="ident")
        make_identity(nc, ident[:])
        ones = stats_pool.tile([P, 1], FP32, tag="ones", name="ones")
        nc.gpsimd.memset(ones[:], 1.0)

        # accumulators for the main loop
        rs_all = stats_pool.tile([P, MB * NB], FP32, tag="rs", name="rs_all")
        e_self = stats_pool.tile([P, MB], FP32, tag="eself", name="e_self")
        pos = stats_pool.tile([P, MB], FP32, tag="pos", name="pos")

        # repsT8[kk][g]: [128, 2048] u16 - partition p = feature pair
        # (kk*256 + 2p, kk*256 + 2p + 1) packed as 2 fp8 bytes; column q =
        # group row (q%16)*128 + q//16 (scratch permutation). fp8 bitcast
        # views give the DoubleRow [128, 2, N] operand APs directly.
        repsT8 = [[repsT_pool.tile([P, CB], U16, tag=f"rT{kk}_{g}",
                                   name=f"repsT8_{kk}_{g}")
                   for g in range(NB)]
                  for kk in range(KK)]

        def rhs_ap(kk, g, ns):
            v = repsT8[kk][g][:].bitcast(FP8).rearrange(
                "p (n two) -> p two n", two=2)
            return v[:, :, ns * 512:(ns + 1) * 512]

        # repsT0[kk]: [128, 2*1024] fp8, plane-slab layout (plane i at cols
        # i*1024..) — LDWEIGHTS rejects the byte-interleaved stride-2 AP, so
        # deinterleave the core's own 1024 columns for the stationary side.
        repsT0_pool = ctx.enter_context(tc.tile_pool(name="repsT0", bufs=1))
        repsT0 = [repsT0_pool.tile([P, 2 * ROWS_PER_CORE], FP8,
                                   tag=f"rT0_{kk}", name=f"repsT0_{kk}")
                  for kk in range(KK)]

        def lhs_ap(kk, m):
            v = repsT0[kk][:].rearrange("p (two m) -> p two m", two=2)
            return v[:, :, m * P:(m + 1) * P]

        # --- issue all row loads up front (DMA runs ahead) -----------------
        rows_g = []
        for g in range(NB):
            rg = rows_pool.tile([P, TPG * D], BF16, tag=f"rows{g}", bufs=1,
                                name=f"rows_{g}")
            rows_g.append(rg)
        TSUB = TPG // LOAD_SPLIT  # row tiles per sub-DMA
        for g in range(NB):
            for s in range(LOAD_SPLIT):
                r0 = g * CB + s * TSUB * P
                src = reps[r0:r0 + TSUB * P, :].rearrange(
                    "(t p) d -> p t d", p=P)
                dst = rows_g[g][:, s * TSUB * D:(s + 1) * TSUB * D].rearrange(
                    "p (t d) -> p t d", d=D)
                nc.sync.dma_start(out=dst, in_=src)

        def phase_a(g):
            n2 = stats_pool.tile([P, TPG], FP32, tag="n2", bufs=2,
                                 name=f"n2_{g}")
            for t in range(TPG):
                sq = sq_pool.tile([P, D], BF16, tag="sq", name=f"sq_{g}_{t}")
                rt = rows_g[g][:, t * D:(t + 1) * D]
                nc.vector.scalar_tensor_tensor(
                    out=sq[:], in0=rt, scalar=1.0, in1=rt,
                    op0=ALU.mult, op1=ALU.mult, accum_out=n2[:, t:t + 1])
            # inv = C * n2 ** -0.5 = exp(-0.5 * ln(n2 / C^2)); Ln+Exp share
            # one ACT table set (forced via _filtered_activation_tables).
            lnn = stats_pool.tile([P, TPG], FP32, tag="lnn", bufs=2,
                                  name=f"lnn_{g}")
            nc.scalar.activation(lnn[:], n2[:], AF.Ln,
                                 scale=1.0 / (C_SCALE * C_SCALE))
            inv = stats_pool.tile([P, TPG], FP32, tag="inv", bufs=2,
                                  name=f"inv_{g}")
            nc.scalar.activation(inv[:], lnn[:], AF.Exp, scale=-0.5)

            normed8 = normed_pool.tile([P, TPG * D], FP8, tag="normed",
                                       name=f"normed_{g}")
            for t in range(TPG):
                nc.vector.tensor_scalar_mul(
                    normed8[:, t * D:(t + 1) * D],
                    rows_g[g][:, t * D:(t + 1) * D], inv[:, t:t + 1])
            # Bounce through DRAM per kk-chunk, permuted so the store writes
            # 4 KiB runs per partition and the transpose read is fully
            # contiguous. Scratch row q = p*16 + t holds group row t*128 + p,
            # so repsT8 column q <-> group row (q%16)*128 + q//16.
            nview16 = normed8[:].bitcast(U16).rearrange(
                "p (t q) -> p t q", q=KK * P)  # [P, TPG, 256]
            for kk in range(KK):
                scr = dram_pool.tile([CB, P], U16, tag=f"scr{kk}_{g}",
                                     name=f"scr_{kk}_{g}")
                nc.sync.dma_start(
                    out=scr[:].rearrange("(p t) c -> p t c", p=P),
                    in_=nview16[:, :, kk * P:(kk + 1) * P])
                nc.sync.dma_start_transpose(repsT8[kk][g][:], scr[:])

        # permuted column q of an et/ps tile holds group row (q%16)*128+q//16,
        # so the columns for m-block rows m*128+j sit at positions 16*j + m.
        def colsel(ap_2d, m):
            return ap_2d.rearrange("p (j s) -> p s j", s=TPG)[:, m, :]

        def phase_b(nb):
            for m in range(MB):
                ps = psum_pool.tile([P, CB], FP32, tag="ps",
                                    name=f"ps_{nb}_{m}")
                for ns in range(NS):
                    for kk in range(KK):
                        nc.tensor.matmul(
                            ps[:, ns * 512:(ns + 1) * 512],
                            lhsT=lhs_ap(kk, m),
                            rhs=rhs_ap(kk, nb, ns),
                            start=(kk == 0), stop=(kk == KK - 1),
                            perf_mode=DR)
                et = exp_pool.tile([P, CB], BF16, tag="et",
                                   name=f"et_{nb}_{m}")
                nc.scalar.activation(
                    et[:], ps[:], AF.Exp, scale=SIM_SCALE,
                    accum_out=rs_all[:, m * NB + nb:m * NB + nb + 1])
                if nb == 0:
                    # self-similarity diagonal: own row m*128+j sits at
                    # permuted column 16*j + m.
                    junk = junk_pool.tile([P, P], FP32, tag="junk",
                                          name=f"junk_s_{m}")
                    nc.vector.scalar_tensor_tensor(
                        out=junk[:], in0=colsel(et[:], m),
                        scalar=1.0, in1=ident[:],
                        op0=ALU.mult, op1=ALU.mult,
                        accum_out=e_self[:, m:m + 1])
                if nb == 2:
                    # positive diagonal: global col 4096+row -> group 2,
                    # same permuted position 16*j + m. ps holds C^2 * sim.
                    junk = junk_pool.tile([P, P], FP32, tag="junk",
                                          name=f"junk_p_{m}")
                    nc.vector.scalar_tensor_tensor(
                        out=junk[:], in0=colsel(ps[:], m),
                        scalar=1.0, in1=ident[:],
                        op0=ALU.mult, op1=ALU.mult,
                        accum_out=pos[:, m:m + 1])

        # interleave: A(g0) B(0) | A(g1) B(1) | ... so every engine queue
        # pipelines (ACT: ln/exp(g) then 8 exps(nb); PE streams while the
        # next group loads/normalizes/transposes).
        for g in range(NB):
            phase_a(g)
            if g == 0:
                # deinterleave + unpermute the core's own 1024 columns into
                # plane-slab lhsT: fp8 index of repsT8 = 2*(16j + s) + i for
                # group row s*128 + j; own rows have s < 8.
                for kk in range(KK):
                    iv = repsT8[kk][0][:].bitcast(FP8).rearrange(
                        "p (j s two) -> p two s j", two=2, s=TPG)
                    ov = repsT0[kk][:].rearrange(
                        "p (two s j) -> p two s j", two=2, s=MB)
                    for i in range(2):
                        nc.vector.tensor_copy(ov[:, i], iv[:, i, :MB, :])
            phase_b(g)

        # --- epilogue ------------------------------------------------------
        sums = epi_pool.tile([P, MB], FP32, tag="sums", name="sums")
        nc.vector.tensor_reduce(
            sums[:], rs_all[:].rearrange("p (m b) -> p m b", b=NB),
            axis=AX.X, op=ALU.add)
        denom = epi_pool.tile([P, MB], FP32, tag="denom", name="denom")
        nc.vector.tensor_sub(denom[:], sums[:], e_self[:])
        ld = epi_pool.tile([P, MB], FP32, tag="ld", name="ld")
        nc.scalar.activation(ld[:], denom[:], AF.Ln)
        # partial = ld - (2/C^2)*pos = (pos * -SIM_SCALE) + ld
        part = epi_pool.tile([P, MB], FP32, tag="part", name="part")
        nc.vector.scalar_tensor_tensor(
            out=part[:], in0=pos[:], scalar=-SIM_SCALE, in1=ld[:],
            op0=ALU.mult, op1=ALU.add)
        rowtot = epi_pool.tile([P, 1], FP32, tag="rowtot", name="rowtot")
        nc.vector.tensor_reduce(rowtot[:], part[:], axis=AX.X, op=ALU.add)
        pfin = psum_pool.tile([P, CB], FP32, tag="ps", name="pfin")
        nc.tensor.matmul(pfin[:1, :1], lhsT=ones[:], rhs=rowtot[:])
        out_sb = epi_pool.tile([1, 1], FP32, tag="osb", name="out_sb")
        nc.vector.tensor_copy(out_sb[:], pfin[:1, :1])
        nc.sync.dma_start(out=out[:, :], in_=out_sb[:])

    with mock.patch("concourse.bacc.get_activation_tables",
                    _filtered_activation_tables):
        nc.compile()
    return nc


_CACHE_LOCK = threading.Lock()
_CACHED_NC = None


def _get_nc():
    global _CACHED_NC
    with _CACHE_LOCK:
        if _CACHED_NC is None:
            _CACHED_NC = _build_kernel()
        return _CACHED_NC


def _run(inputs, trace=False):
    z_i = np.asarray(inputs["z_i"], dtype=np.float32)
    z_j = np.asarray(inputs["z_j"], dtype=np.float32)
    reps = np.concatenate([z_i, z_j], axis=0).astype(ml_dtypes.bfloat16)
    in_maps = [
        {"reps": np.ascontiguousarray(
            np.roll(reps, -ROWS_PER_CORE * i, axis=0))}
        for i in range(N_CORES)
    ]
    nc = _get_nc()
    res = run_bass_kernel_spmd(nc, in_maps, list(range(N_CORES)), trace=trace)
    partials = [float(res.results[i]["out"][0, 0]) for i in range(N_CORES)]
    loss = np.float32(np.sum(np.asarray(partials, dtype=np.float64)) / TWO_N)
    return loss, res


def kernel(**inputs):
    loss, _ = _run(inputs, trace=False)
    return np.asarray(loss, dtype=np.float32)
